# revision 11
# baseline (speedup 1.0000x reference)
"""ArcFace loss on 8 TRN2 NeuronCores — class-axis (vocab) parallel.

Full inputs in, full scalar loss out. Classes sharded 12500/core; x and the
gathered target weight rows are replicated. Per-core: normalize, bf16 matmul
x_norm @ w_normT, fused exp(s*cos(theta+m)) epilogue with row-sum
accumulation, one 8KB AllReduce of the row sums, then an exact f32
target-term correction + log + mean computed redundantly on every core.
"""

import math

import numpy as np

import concourse.bass as bass
import concourse.tile as tile
from concourse import bacc, masks, mybir
from concourse.bass_utils import run_bass_kernel_spmd

B = 2048
D = 128
C = 100000
NCORES = 8
CS = C // NCORES          # 12500 classes per core
NBT = B // 128            # 16 batch tiles
NWT = (CS + 127) // 128   # 98 class tiles (last one is 84 rows)
WTAIL = CS - (NWT - 1) * 128  # 84
CHUNK = 500               # main-loop free-dim chunk (fits one PSUM bank)
NCK = CS // CHUNK         # 25

MARGIN = 0.5
SCALE = 70.0
COS_M = math.cos(MARGIN)
SIN_M = math.sin(MARGIN)
MM = math.sin(math.pi - MARGIN) * MARGIN
K1 = SCALE * COS_M        # 61.43...
K2 = SCALE * SIN_M        # 33.56...
K2SQ = K2 * K2

F32 = mybir.dt.float32
F16 = mybir.dt.float16
BF16 = mybir.dt.bfloat16
AF = mybir.ActivationFunctionType
ALU = mybir.AluOpType

_NC = None


def _build():
    nc = bacc.Bacc(
        "TRN2", target_bir_lowering=False, debug=False, num_devices=NCORES)
    x_ext = nc.declare_dram_parameter("x", [B, D], F32, isOutput=False)
    w_ext = nc.declare_dram_parameter("w", [CS, D], F32, isOutput=False)
    wtg_ext = nc.declare_dram_parameter("wtg", [B, D], F32, isOutput=False)
    out_ext = nc.declare_dram_parameter("out", [1, 1], F32, isOutput=True)

    with tile.TileContext(nc) as tc:
        from contextlib import ExitStack

        with ExitStack() as ctx:
            singles = ctx.enter_context(tc.tile_pool(name="singles", bufs=1))
            scr = ctx.enter_context(tc.tile_pool(name="scr", bufs=3))
            mm_pool = ctx.enter_context(
                tc.tile_pool(name="mm", bufs=4, space="PSUM"))
            tp_pool = ctx.enter_context(
                tc.tile_pool(name="tp", bufs=2, space="PSUM"))
            fin_pool = ctx.enter_context(
                tc.tile_pool(name="fin", bufs=1, space="PSUM"))
            dram = ctx.enter_context(
                tc.tile_pool(name="dram", bufs=1, space="DRAM"))

            ident = singles.tile([128, 128], BF16)
            masks.make_identity(nc, ident[:])
            ones = singles.tile([128, 1], F32)
            nc.gpsimd.memset(ones[:], 1.0)
            k2sqb = singles.tile([128, 1], F32)
            nc.gpsimd.memset(k2sqb[:], K2SQ)

            # ---------------- load x / wtg (replicated) ----------------
            x_all = singles.tile([128, B], F32)      # col bt*128+d
            nc.sync.dma_start(
                out=x_all[:].rearrange("p (a d) -> p a d", d=D),
                in_=x_ext.rearrange("(a p) d -> p a d", p=128))
            wtg_all = singles.tile([128, B], F32)
            nc.sync.dma_start(
                out=wtg_all[:].rearrange("p (a d) -> p a d", d=D),
                in_=wtg_ext.rearrange("(a p) d -> p a d", p=128))

            # ---------------- load w shard (scoped pool, freed later) ----
            NFULL = NWT - 1  # 97 full 128-row tiles
            wload_ctx = ExitStack()
            wload = wload_ctx.enter_context(
                tc.tile_pool(name="wload", bufs=1))
            w_all = wload.tile([128, NFULL * 128], F32)
            rows_per_dma = 12 * 128
            ndma = (NFULL * 128) // rows_per_dma  # 8 DMAs of 1536 rows
            for i in range(ndma):
                r0 = i * rows_per_dma
                nc.sync.dma_start(
                    out=w_all[:, r0:r0 + rows_per_dma].rearrange(
                        "p (a d) -> p a d", d=D),
                    in_=w_ext[r0:r0 + rows_per_dma, :].rearrange(
                        "(a p) d -> p a d", p=128))
            rem0 = ndma * rows_per_dma
            if rem0 < NFULL * 128:
                nc.sync.dma_start(
                    out=w_all[:, rem0:NFULL * 128].rearrange(
                        "p (a d) -> p a d", d=D),
                    in_=w_ext[rem0:NFULL * 128, :].rearrange(
                        "(a p) d -> p a d", p=128))
            w_tail = wload.tile([128, 128], F32)
            nc.sync.dma_start(
                out=w_tail[:WTAIL, :], in_=w_ext[NFULL * 128:CS, :])

            # ---------------- normalize w, build wnT [128, CS] bf16 ------
            ns2 = singles.tile([128, NWT], F32)
            for t in range(NWT):
                if t < NFULL:
                    wt = w_all[:, t * 128:(t + 1) * 128]
                    psz = 128
                else:
                    psz = WTAIL
                    wt = w_tail[:psz, :]
                sq = scr.tile([128, 128], F32, tag="sq")
                nc.vector.scalar_tensor_tensor(
                    out=sq[:psz, :], in0=wt, scalar=1.0, in1=wt,
                    op0=ALU.mult, op1=ALU.mult,
                    accum_out=ns2[:psz, t:t + 1])
            wnrm = singles.tile([128, NWT], F32)
            nc.scalar.activation(wnrm[:], ns2[:], AF.Sqrt)
            winv = singles.tile([128, NWT], F32)
            nc.vector.reciprocal(winv[:], wnrm[:])

            wnT = singles.tile([128, CS], BF16)
            for t in range(NWT):
                if t < NFULL:
                    wt = w_all[:, t * 128:(t + 1) * 128]
                    psz = 128
                else:
                    psz = WTAIL
                    wt = w_tail[:psz, :]
                wn = scr.tile([128, 128], BF16, tag="wn")
                nc.vector.tensor_scalar(
                    out=wn[:psz, :], in0=wt, scalar1=winv[:psz, t:t + 1],
                    scalar2=None, op0=ALU.mult)
                tp = tp_pool.tile([128, 128], BF16)
                nc.tensor.transpose(tp[:, :psz], wn[:psz, :], ident[:psz, :psz])
                nc.scalar.activation(
                    wnT[:, t * 128:t * 128 + psz], tp[:, :psz], AF.Copy)

            wload_ctx.close()

            # ---------------- normalize x, build xnT [128, B] bf16 -------
            xs2 = singles.tile([128, NBT], F32)
            for t in range(NBT):
                xt = x_all[:, t * 128:(t + 1) * 128]
                sq = scr.tile([128, 128], F32, tag="sq")
                nc.vector.scalar_tensor_tensor(
                    out=sq[:], in0=xt, scalar=1.0, in1=xt,
                    op0=ALU.mult, op1=ALU.mult, accum_out=xs2[:, t:t + 1])
            xnrm = singles.tile([128, NBT], F32)
            nc.scalar.activation(xnrm[:], xs2[:], AF.Sqrt)
            xinv = singles.tile([128, NBT], F32)
            nc.vector.reciprocal(xinv[:], xnrm[:])

            xnT = singles.tile([128, B], BF16)
            for t in range(NBT):
                xt = x_all[:, t * 128:(t + 1) * 128]
                xn = scr.tile([128, 128], BF16, tag="wn")
                nc.vector.tensor_scalar(
                    out=xn[:], in0=xt, scalar1=xinv[:, t:t + 1],
                    scalar2=None, op0=ALU.mult)
                tp = tp_pool.tile([128, 128], BF16)
                nc.tensor.transpose(tp[:], xn[:], ident[:])
                nc.scalar.activation(
                    xnT[:, t * 128:(t + 1) * 128], tp[:], AF.Copy)

            # ---------------- main loop: cos -> exp -> row partial sums --
            # Phase-batched per half-row-block so ACT runs one wide Sqrt and
            # one wide Exp per group (avoids per-chunk act-table reloads).
            GROUPS = [(0, 13), (13, 25)]
            GW = 13 * CHUNK  # 6500, max group width
            cg = ctx.enter_context(tc.tile_pool(name="cg", bufs=2))
            ug = ctx.enter_context(tc.tile_pool(name="ug", bufs=2))
            vg = ctx.enter_context(tc.tile_pool(name="vg", bufs=2))
            ag = ctx.enter_context(tc.tile_pool(name="ag", bufs=2))
            eg = ctx.enter_context(tc.tile_pool(name="eg", bufs=1))
            rs2 = singles.tile([128, NBT * 2], F32)
            for bt in range(NBT):
                lhsT = xnT[:, bt * 128:(bt + 1) * 128]
                for g, (c0, c1) in enumerate(GROUPS):
                    W = (c1 - c0) * CHUNK
                    c_all = cg.tile([128, GW], BF16, tag="c")
                    for ck in range(c0, c1):
                        pc = mm_pool.tile([128, CHUNK], F32)
                        nc.tensor.matmul(
                            pc[:], lhsT, wnT[:, ck * CHUNK:(ck + 1) * CHUNK],
                            start=True, stop=True)
                        o = (ck - c0) * CHUNK
                        nc.vector.tensor_copy(c_all[:, o:o + CHUNK], pc[:])
                    u_all = ug.tile([128, GW], BF16, tag="u")
                    nc.vector.scalar_tensor_tensor(
                        out=u_all[:, :W], in0=c_all[:, :W], scalar=1.0,
                        in1=c_all[:, :W], op0=ALU.mult, op1=ALU.mult)
                    v_all = vg.tile([128, GW], BF16, tag="v")
                    nc.scalar.activation(
                        v_all[:, :W], u_all[:, :W], AF.Sqrt,
                        scale=-K2SQ, bias=k2sqb[:])
                    a_all = ag.tile([128, GW], BF16, tag="a")
                    nc.vector.scalar_tensor_tensor(
                        out=a_all[:, :W], in0=c_all[:, :W], scalar=K1,
                        in1=v_all[:, :W], op0=ALU.mult, op1=ALU.subtract)
                    e_all = eg.tile([128, GW], BF16, tag="e")
                    col = bt * 2 + g
                    nc.scalar.activation(
                        e_all[:, :W], a_all[:, :W], AF.Exp,
                        accum_out=rs2[:, col:col + 1])

            rs = singles.tile([128, NBT], F32)
            rs2v = rs2[:].rearrange("p (a two) -> p a two", two=2)
            nc.vector.tensor_tensor(
                rs[:], rs2v[:, :, 0], rs2v[:, :, 1], op=ALU.add)

            # ---------------- AllReduce row sums (8KB) -------------------
            rs_in = dram.tile([128, NBT], F32)
            rs_out = dram.tile([128, NBT], F32)
            nc.sync.dma_start(rs_in[:], rs[:])
            nc.gpsimd.collective_compute(
                "AllReduce", ALU.add,
                replica_groups=[list(range(NCORES))],
                ins=[rs_in.opt()], outs=[rs_out.opt()])
            rsum = singles.tile([128, NBT], F32)
            nc.sync.dma_start(rsum[:], rs_out[:])

            # ---------------- exact target-term correction (f32) ---------
            ws2 = singles.tile([128, NBT], F32)
            dots = singles.tile([128, NBT], F32)
            for t in range(NBT):
                gt = wtg_all[:, t * 128:(t + 1) * 128]
                xt = x_all[:, t * 128:(t + 1) * 128]
                sq = scr.tile([128, 128], F32, tag="sq")
                nc.vector.scalar_tensor_tensor(
                    out=sq[:], in0=gt, scalar=1.0, in1=gt,
                    op0=ALU.mult, op1=ALU.mult, accum_out=ws2[:, t:t + 1])
                dt_ = scr.tile([128, 128], F32, tag="sq")
                nc.vector.tensor_tensor(dt_[:], gt, xt, op=ALU.mult)
                nc.vector.tensor_reduce(
                    dots[:, t:t + 1], dt_[:], axis=mybir.AxisListType.XYZW,
                    op=ALU.add)
            wgn = singles.tile([128, NBT], F32)
            nc.scalar.activation(wgn[:], ws2[:], AF.Sqrt)
            wgi = singles.tile([128, NBT], F32)
            nc.vector.reciprocal(wgi[:], wgn[:])

            ct0 = singles.tile([128, NBT], F32)
            nc.vector.tensor_tensor(ct0[:], dots[:], xinv[:], op=ALU.mult)
            ct = singles.tile([128, NBT], F32)
            nc.vector.tensor_tensor(ct[:], ct0[:], wgi[:], op=ALU.mult)

            u2 = singles.tile([128, NBT], F32)
            nc.vector.tensor_tensor(u2[:], ct[:], ct[:], op=ALU.mult)
            v2 = singles.tile([128, NBT], F32)
            nc.scalar.activation(v2[:], u2[:], AF.Sqrt, scale=-K2SQ,
                                 bias=k2sqb[:])
            a2 = singles.tile([128, NBT], F32)
            nc.vector.scalar_tensor_tensor(
                out=a2[:], in0=ct[:], scalar=K1, in1=v2[:],
                op0=ALU.mult, op1=ALU.subtract)
            t1 = singles.tile([128, NBT], F32)
            nc.scalar.activation(t1[:], a2[:], AF.Exp)
            a3 = singles.tile([128, NBT], F32)
            nc.vector.tensor_scalar(
                out=a3[:], in0=ct[:], scalar1=SCALE, scalar2=-SCALE * MM,
                op0=ALU.mult, op1=ALU.add)
            t2 = singles.tile([128, NBT], F32)
            nc.scalar.activation(t2[:], a3[:], AF.Exp)

            s0 = singles.tile([128, NBT], F32)
            nc.vector.tensor_tensor(s0[:], rsum[:], t1[:], op=ALU.subtract)
            s1 = singles.tile([128, NBT], F32)
            nc.vector.tensor_tensor(s1[:], s0[:], t2[:], op=ALU.add)
            lse = singles.tile([128, NBT], F32)
            nc.scalar.activation(lse[:], s1[:], AF.Ln)
            loss = singles.tile([128, NBT], F32)
            nc.vector.tensor_tensor(loss[:], lse[:], a3[:], op=ALU.subtract)

            lscr = singles.tile([128, NBT], F32)
            lcol = singles.tile([128, 1], F32)
            nc.scalar.activation(
                lscr[:], loss[:], AF.Identity, scale=1.0 / B,
                accum_out=lcol[:])
            fin = fin_pool.tile([1, 1], F32)
            nc.tensor.matmul(fin[:1, :1], ones[:], lcol[:],
                             start=True, stop=True)
            out_sb = singles.tile([1, 1], F32)
            nc.scalar.activation(out_sb[:1, :1], fin[:1, :1], AF.Copy)
            nc.sync.dma_start(out_ext[:, :], out_sb[:1, :1])

    nc.finalize()
    return nc


def _get_nc():
    global _NC
    if _NC is None:
        _NC = _build()
    return _NC


def _in_maps(inputs):
    x = np.ascontiguousarray(np.asarray(inputs["x"], dtype=np.float32))
    target = np.asarray(inputs["target"]).astype(np.int64)
    weight = np.ascontiguousarray(
        np.asarray(inputs["weight"], dtype=np.float32))
    wtg = np.ascontiguousarray(weight[target])
    maps = []
    for c in range(NCORES):
        shard = np.ascontiguousarray(weight[c * CS:(c + 1) * CS])
        maps.append({"x": x, "w": shard, "wtg": wtg})
    return maps


def run(inputs, trace=False, **kw):
    res = run_bass_kernel_spmd(
        _get_nc(), _in_maps(inputs), core_ids=list(range(NCORES)),
        trace=trace, **kw)
    out = np.asarray(res.results[0]["out"], dtype=np.float32).reshape(())
    return out, res


def kernel(**inputs):
    out, _ = run(inputs, trace=False)
    return out


# revision 14
# speedup vs baseline: 1.1629x; 1.1629x over previous
"""ArcFace loss on 8 TRN2 NeuronCores — class-axis (vocab) parallel.

Full inputs in, full scalar loss out. Classes sharded 12500/core; x and the
gathered target weight rows are replicated. Per-core: normalize, bf16 matmul
x_norm @ w_normT, fused exp(s*cos(theta+m)) epilogue with row-sum
accumulation, one 8KB AllReduce of the row sums, then an exact f32
target-term correction + log + mean computed redundantly on every core.
"""

import math

import numpy as np

import concourse.bass as bass
import concourse.tile as tile
from concourse import bacc, masks, mybir
from concourse.bass_utils import run_bass_kernel_spmd

B = 2048
D = 128
C = 100000
NCORES = 8
CS = C // NCORES          # 12500 classes per core
NBT = B // 128            # 16 batch tiles
NWT = (CS + 127) // 128   # 98 class tiles (last one is 84 rows)
WTAIL = CS - (NWT - 1) * 128  # 84
CHUNK = 500               # main-loop free-dim chunk (fits one PSUM bank)
NCK = CS // CHUNK         # 25

MARGIN = 0.5
SCALE = 70.0
COS_M = math.cos(MARGIN)
SIN_M = math.sin(MARGIN)
MM = math.sin(math.pi - MARGIN) * MARGIN
K1 = SCALE * COS_M        # 61.43...
K2 = SCALE * SIN_M        # 33.56...
K2SQ = K2 * K2

F32 = mybir.dt.float32
F16 = mybir.dt.float16
BF16 = mybir.dt.bfloat16
AF = mybir.ActivationFunctionType
ALU = mybir.AluOpType

_NC = None


def _build():
    nc = bacc.Bacc(
        "TRN2", target_bir_lowering=False, debug=False, num_devices=NCORES)
    x_ext = nc.declare_dram_parameter("x", [B, D], F32, isOutput=False)
    w_ext = nc.declare_dram_parameter("w", [CS, D], F32, isOutput=False)
    wtg_ext = nc.declare_dram_parameter("wtg", [B, D], F32, isOutput=False)
    out_ext = nc.declare_dram_parameter("out", [1, 1], F32, isOutput=True)

    with tile.TileContext(nc) as tc:
        from contextlib import ExitStack

        with ExitStack() as ctx:
            singles = ctx.enter_context(tc.tile_pool(name="singles", bufs=1))
            scr = ctx.enter_context(tc.tile_pool(name="scr", bufs=3))
            mm_pool = ctx.enter_context(
                tc.tile_pool(name="mm", bufs=4, space="PSUM"))
            tp_pool = ctx.enter_context(
                tc.tile_pool(name="tp", bufs=2, space="PSUM"))
            fin_pool = ctx.enter_context(
                tc.tile_pool(name="fin", bufs=1, space="PSUM"))
            dram = ctx.enter_context(
                tc.tile_pool(name="dram", bufs=2, space="DRAM"))

            ident = singles.tile([128, 128], BF16)
            masks.make_identity(nc, ident[:])
            ones = singles.tile([128, 1], F32)
            nc.gpsimd.memset(ones[:], 1.0)
            k2sqb = singles.tile([128, 1], F32)
            nc.gpsimd.memset(k2sqb[:], K2SQ)

            # ---------------- load x / wtg (replicated) ----------------
            x_all = singles.tile([128, B], F32)      # col bt*128+d
            nc.sync.dma_start(
                out=x_all[:].rearrange("p (a d) -> p a d", d=D),
                in_=x_ext.rearrange("(a p) d -> p a d", p=128))
            wtg_all = singles.tile([128, B], F32)
            nc.sync.dma_start(
                out=wtg_all[:].rearrange("p (a d) -> p a d", d=D),
                in_=wtg_ext.rearrange("(a p) d -> p a d", p=128))

            # ---------------- load w shard (scoped pool, freed later) ----
            NFULL = NWT - 1  # 97 full 128-row tiles
            wload_ctx = ExitStack()
            wload = wload_ctx.enter_context(
                tc.tile_pool(name="wload", bufs=1))
            w_all = wload.tile([128, NFULL * 128], F32)
            rows_per_dma = 12 * 128
            ndma = (NFULL * 128) // rows_per_dma  # 8 DMAs of 1536 rows
            for i in range(ndma):
                r0 = i * rows_per_dma
                nc.sync.dma_start(
                    out=w_all[:, r0:r0 + rows_per_dma].rearrange(
                        "p (a d) -> p a d", d=D),
                    in_=w_ext[r0:r0 + rows_per_dma, :].rearrange(
                        "(a p) d -> p a d", p=128))
            rem0 = ndma * rows_per_dma
            if rem0 < NFULL * 128:
                nc.sync.dma_start(
                    out=w_all[:, rem0:NFULL * 128].rearrange(
                        "p (a d) -> p a d", d=D),
                    in_=w_ext[rem0:NFULL * 128, :].rearrange(
                        "(a p) d -> p a d", p=128))
            w_tail = wload.tile([128, 128], F32)
            nc.sync.dma_start(
                out=w_tail[:WTAIL, :], in_=w_ext[NFULL * 128:CS, :])

            # ---------------- normalize w, build wnT [128, CS] bf16 ------
            ns2 = singles.tile([128, NWT], F32)
            for t in range(NWT):
                if t < NFULL:
                    wt = w_all[:, t * 128:(t + 1) * 128]
                    psz = 128
                else:
                    psz = WTAIL
                    wt = w_tail[:psz, :]
                sq = scr.tile([128, 128], F32, tag="sq")
                nc.vector.scalar_tensor_tensor(
                    out=sq[:psz, :], in0=wt, scalar=1.0, in1=wt,
                    op0=ALU.mult, op1=ALU.mult,
                    accum_out=ns2[:psz, t:t + 1])
            wnrm = singles.tile([128, NWT], F32)
            winv = singles.tile([128, NWT], F32)
            for i in range(0, NWT, 12):
                j = min(i + 12, NWT)
                nc.scalar.activation(wnrm[:, i:j], ns2[:, i:j], AF.Sqrt)
                nc.vector.reciprocal(winv[:, i:j], wnrm[:, i:j])

            wnT = singles.tile([128, CS], BF16)
            for t in range(NWT):
                if t < NFULL:
                    wt = w_all[:, t * 128:(t + 1) * 128]
                    psz = 128
                else:
                    psz = WTAIL
                    wt = w_tail[:psz, :]
                wn = scr.tile([128, 128], BF16, tag="wn")
                nc.vector.tensor_scalar(
                    out=wn[:psz, :], in0=wt, scalar1=winv[:psz, t:t + 1],
                    scalar2=None, op0=ALU.mult)
                tp = tp_pool.tile([128, 128], BF16)
                nc.tensor.transpose(tp[:, :psz], wn[:psz, :], ident[:psz, :psz])
                nc.scalar.activation(
                    wnT[:, t * 128:t * 128 + psz], tp[:, :psz], AF.Copy)

            wload_ctx.close()

            # ---------------- normalize x, build xnT [128, B] bf16 -------
            xs2 = singles.tile([128, NBT], F32)
            for t in range(NBT):
                xt = x_all[:, t * 128:(t + 1) * 128]
                sq = scr.tile([128, 128], F32, tag="sq")
                nc.vector.scalar_tensor_tensor(
                    out=sq[:], in0=xt, scalar=1.0, in1=xt,
                    op0=ALU.mult, op1=ALU.mult, accum_out=xs2[:, t:t + 1])
            xnrm = singles.tile([128, NBT], F32)
            nc.scalar.activation(xnrm[:], xs2[:], AF.Sqrt)
            xinv = singles.tile([128, NBT], F32)
            nc.vector.reciprocal(xinv[:], xnrm[:])

            xnT = singles.tile([128, B], BF16)
            for t in range(NBT):
                xt = x_all[:, t * 128:(t + 1) * 128]
                xn = scr.tile([128, 128], BF16, tag="wn")
                nc.vector.tensor_scalar(
                    out=xn[:], in0=xt, scalar1=xinv[:, t:t + 1],
                    scalar2=None, op0=ALU.mult)
                tp = tp_pool.tile([128, 128], BF16)
                nc.tensor.transpose(tp[:], xn[:], ident[:])
                nc.scalar.activation(
                    xnT[:, t * 128:(t + 1) * 128], tp[:], AF.Copy)

            # ---------------- main loop: cos -> exp -> row partial sums --
            # Phase-batched; ACT ops paired per bt (sqrt,sqrt,exp,exp) to
            # minimize act-table reloads; STT passes split DVE/GPSIMD.
            GROUPS = [(0, 13), (13, 25)]
            GW = 13 * CHUNK  # 6500, max group width
            cg = ctx.enter_context(tc.tile_pool(name="cg", bufs=3))
            ug = ctx.enter_context(tc.tile_pool(name="ug", bufs=2))
            vg = ctx.enter_context(tc.tile_pool(name="vg", bufs=2))
            ag = ctx.enter_context(tc.tile_pool(name="ag", bufs=2))
            eg = ctx.enter_context(tc.tile_pool(name="eg", bufs=1))
            rs2 = singles.tile([128, NBT * 2], F32)
            for bt in range(NBT):
                lhsT = xnT[:, bt * 128:(bt + 1) * 128]
                cs, us, vs, as_ = [], [], [], []
                for g, (c0, c1) in enumerate(GROUPS):
                    W = (c1 - c0) * CHUNK
                    c_all = cg.tile([128, GW], F16, tag="c")
                    for ck in range(c0, c1):
                        pc = mm_pool.tile([128, CHUNK], F32)
                        nc.tensor.matmul(
                            pc[:], lhsT, wnT[:, ck * CHUNK:(ck + 1) * CHUNK],
                            start=True, stop=True)
                        o = (ck - c0) * CHUNK
                        nc.vector.tensor_copy(c_all[:, o:o + CHUNK], pc[:])
                    u_all = ug.tile([128, GW], F16, tag="u")
                    if g == 0:
                        nc.vector.scalar_tensor_tensor(
                            out=u_all[:, :W], in0=c_all[:, :W], scalar=1.0,
                            in1=c_all[:, :W], op0=ALU.mult, op1=ALU.mult)
                    else:
                        nc.scalar.activation(
                            u_all[:, :W], c_all[:, :W], AF.Square)
                    cs.append((c_all, W)); us.append(u_all)
                for g in range(2):
                    (c_all, W), u_all = cs[g], us[g]
                    v_all = vg.tile([128, GW], F16, tag="v")
                    nc.scalar.activation(
                        v_all[:, :W], u_all[:, :W], AF.Sqrt,
                        scale=-K2SQ, bias=k2sqb[:])
                    vs.append(v_all)
                for g in range(2):
                    (c_all, W), v_all = cs[g], vs[g]
                    a_all = ag.tile([128, GW], F16, tag="a")
                    nc.vector.scalar_tensor_tensor(
                        out=a_all[:, :W], in0=c_all[:, :W], scalar=K1,
                        in1=v_all[:, :W], op0=ALU.mult, op1=ALU.subtract)
                    as_.append(a_all)
                for g in range(2):
                    (c_all, W), a_all = cs[g], as_[g]
                    e_all = eg.tile([128, GW], BF16, tag="e")
                    col = bt * 2 + g
                    nc.scalar.activation(
                        e_all[:, :W], a_all[:, :W], AF.Exp,
                        accum_out=rs2[:, col:col + 1])

            # ---------------- AllReduce row sums, split in halves --------
            rs = singles.tile([128, NBT], F32)
            rs2v = rs2[:].rearrange("p (a two) -> p a two", two=2)
            H = NBT // 2
            rsum = singles.tile([128, NBT], F32)
            rs_in0 = dram.tile([128, H], F32, tag="rsin")
            rs_out0 = dram.tile([128, H], F32, tag="rsout")
            rs_in1 = dram.tile([128, H], F32, tag="rsin")
            rs_out1 = dram.tile([128, H], F32, tag="rsout")
            nc.vector.tensor_tensor(
                rs[:, :H], rs2v[:, :H, 0], rs2v[:, :H, 1], op=ALU.add)
            nc.sync.dma_start(rs_in0[:], rs[:, :H])
            nc.gpsimd.collective_compute(
                "AllReduce", ALU.add,
                replica_groups=[list(range(NCORES))],
                ins=[rs_in0.opt()], outs=[rs_out0.opt()])
            nc.sync.dma_start(rsum[:, :H], rs_out0[:])
            nc.vector.tensor_tensor(
                rs[:, H:], rs2v[:, H:, 0], rs2v[:, H:, 1], op=ALU.add)
            nc.sync.dma_start(rs_in1[:], rs[:, H:])
            nc.gpsimd.collective_compute(
                "AllReduce", ALU.add,
                replica_groups=[list(range(NCORES))],
                ins=[rs_in1.opt()], outs=[rs_out1.opt()])
            nc.sync.dma_start(rsum[:, H:], rs_out1[:])

            # ---------------- exact target-term correction (f32) ---------
            ws2 = singles.tile([128, NBT], F32)
            dots = singles.tile([128, NBT], F32)
            for t in range(NBT):
                gt = wtg_all[:, t * 128:(t + 1) * 128]
                xt = x_all[:, t * 128:(t + 1) * 128]
                sq = scr.tile([128, 128], F32, tag="sq")
                nc.vector.scalar_tensor_tensor(
                    out=sq[:], in0=gt, scalar=1.0, in1=gt,
                    op0=ALU.mult, op1=ALU.mult, accum_out=ws2[:, t:t + 1])
                dt_ = scr.tile([128, 128], F32, tag="sq")
                nc.vector.tensor_tensor(dt_[:], gt, xt, op=ALU.mult)
                nc.vector.tensor_reduce(
                    dots[:, t:t + 1], dt_[:], axis=mybir.AxisListType.XYZW,
                    op=ALU.add)
            wgn = singles.tile([128, NBT], F32)
            nc.scalar.activation(wgn[:], ws2[:], AF.Sqrt)
            wgi = singles.tile([128, NBT], F32)
            nc.vector.reciprocal(wgi[:], wgn[:])

            ct0 = singles.tile([128, NBT], F32)
            nc.vector.tensor_tensor(ct0[:], dots[:], xinv[:], op=ALU.mult)
            ct = singles.tile([128, NBT], F32)
            nc.vector.tensor_tensor(ct[:], ct0[:], wgi[:], op=ALU.mult)

            u2 = singles.tile([128, NBT], F32)
            nc.vector.tensor_tensor(u2[:], ct[:], ct[:], op=ALU.mult)
            v2 = singles.tile([128, NBT], F32)
            nc.scalar.activation(v2[:], u2[:], AF.Sqrt, scale=-K2SQ,
                                 bias=k2sqb[:])
            a2 = singles.tile([128, NBT], F32)
            nc.vector.scalar_tensor_tensor(
                out=a2[:], in0=ct[:], scalar=K1, in1=v2[:],
                op0=ALU.mult, op1=ALU.subtract)
            t1 = singles.tile([128, NBT], F32)
            nc.scalar.activation(t1[:], a2[:], AF.Exp)
            a3 = singles.tile([128, NBT], F32)
            nc.vector.tensor_scalar(
                out=a3[:], in0=ct[:], scalar1=SCALE, scalar2=-SCALE * MM,
                op0=ALU.mult, op1=ALU.add)
            t2 = singles.tile([128, NBT], F32)
            nc.scalar.activation(t2[:], a3[:], AF.Exp)

            s0 = singles.tile([128, NBT], F32)
            nc.vector.tensor_tensor(s0[:], rsum[:], t1[:], op=ALU.subtract)
            s1 = singles.tile([128, NBT], F32)
            nc.vector.tensor_tensor(s1[:], s0[:], t2[:], op=ALU.add)
            lse = singles.tile([128, NBT], F32)
            nc.scalar.activation(lse[:], s1[:], AF.Ln)
            loss = singles.tile([128, NBT], F32)
            nc.vector.tensor_tensor(loss[:], lse[:], a3[:], op=ALU.subtract)

            lscr = singles.tile([128, NBT], F32)
            lcol = singles.tile([128, 1], F32)
            nc.scalar.activation(
                lscr[:], loss[:], AF.Identity, scale=1.0 / B,
                accum_out=lcol[:])
            fin = fin_pool.tile([1, 1], F32)
            nc.tensor.matmul(fin[:1, :1], ones[:], lcol[:],
                             start=True, stop=True)
            out_sb = singles.tile([1, 1], F32)
            nc.scalar.activation(out_sb[:1, :1], fin[:1, :1], AF.Copy)
            nc.sync.dma_start(out_ext[:, :], out_sb[:1, :1])

    nc.finalize()
    return nc


def _get_nc():
    global _NC
    if _NC is None:
        _NC = _build()
    return _NC


def _in_maps(inputs):
    x = np.ascontiguousarray(np.asarray(inputs["x"], dtype=np.float32))
    target = np.asarray(inputs["target"]).astype(np.int64)
    weight = np.ascontiguousarray(
        np.asarray(inputs["weight"], dtype=np.float32))
    wtg = np.ascontiguousarray(weight[target])
    maps = []
    for c in range(NCORES):
        shard = np.ascontiguousarray(weight[c * CS:(c + 1) * CS])
        maps.append({"x": x, "w": shard, "wtg": wtg})
    return maps


def run(inputs, trace=False, **kw):
    res = run_bass_kernel_spmd(
        _get_nc(), _in_maps(inputs), core_ids=list(range(NCORES)),
        trace=trace, **kw)
    out = np.asarray(res.results[0]["out"], dtype=np.float32).reshape(())
    return out, res


def kernel(**inputs):
    out, _ = run(inputs, trace=False)
    return out


# revision 15
# speedup vs baseline: 1.3892x; 1.1946x over previous
"""ArcFace loss on 8 TRN2 NeuronCores — class-axis (vocab) parallel.

Full inputs in, full scalar loss out. Classes sharded 12500/core; x and the
gathered target weight rows are replicated. Per-core: normalize, bf16 matmul
x_norm @ w_normT, phase-batched exp(s*cos(theta+m)) epilogue with row-sum
accumulation (u-pass split DVE/ACT for engine balance, ACT ops ordered to
minimize act-table reloads), two half AllReduces of the row sums (first one
overlaps the second half of compute), then an exact f32 target-term
correction + log + mean computed redundantly on every core.
"""

import math
from contextlib import ExitStack

import numpy as np

import concourse.bass as bass
import concourse.tile as tile
from concourse import bacc, masks, mybir
from concourse.bass_utils import run_bass_kernel_spmd

B = 2048
D = 128
C = 100000
NCORES = 8
CS = C // NCORES          # 12500 classes per core
NBT = B // 128            # 16 batch tiles
NWT = (CS + 127) // 128   # 98 class tiles (last one is 84 rows)
WTAIL = CS - (NWT - 1) * 128  # 84
CHUNK = 500               # main-loop free-dim chunk (fits one PSUM bank)
NCK = CS // CHUNK         # 25

MARGIN = 0.5
SCALE = 70.0
COS_M = math.cos(MARGIN)
SIN_M = math.sin(MARGIN)
MM = math.sin(math.pi - MARGIN) * MARGIN
K1 = SCALE * COS_M        # 61.43...
K2 = SCALE * SIN_M        # 33.56...
K2SQ = K2 * K2

F32 = mybir.dt.float32
F16 = mybir.dt.float16
BF16 = mybir.dt.bfloat16
AF = mybir.ActivationFunctionType
ALU = mybir.AluOpType

_NC = None


def _build():
    nc = bacc.Bacc(
        "TRN2", target_bir_lowering=False, debug=False, num_devices=NCORES)
    x_ext = nc.declare_dram_parameter("x", [B, D], F32, isOutput=False)
    w_ext = nc.declare_dram_parameter("w", [CS, D], F32, isOutput=False)
    wtg_ext = nc.declare_dram_parameter("wtg", [B, D], F32, isOutput=False)
    out_ext = nc.declare_dram_parameter("out", [1, 1], F32, isOutput=True)

    with tile.TileContext(nc) as tc:
        with ExitStack() as ctx:
            singles = ctx.enter_context(tc.tile_pool(name="singles", bufs=1))
            scr = ctx.enter_context(tc.tile_pool(name="scr", bufs=3))
            mm_pool = ctx.enter_context(
                tc.tile_pool(name="mm", bufs=4, space="PSUM"))
            tp_pool = ctx.enter_context(
                tc.tile_pool(name="tp", bufs=2, space="PSUM"))
            fin_pool = ctx.enter_context(
                tc.tile_pool(name="fin", bufs=1, space="PSUM"))
            dram = ctx.enter_context(
                tc.tile_pool(name="dram", bufs=2, space="DRAM"))

            ident = singles.tile([128, 128], BF16)
            masks.make_identity(nc, ident[:])
            ones = singles.tile([128, 1], F32)
            nc.gpsimd.memset(ones[:], 1.0)
            k2sqb = singles.tile([128, 1], F32)
            nc.gpsimd.memset(k2sqb[:], K2SQ)

            # ---------------- x path first (short pole to first matmul) --
            x_all = singles.tile([128, B], F32)      # col bt*128+d
            nc.sync.dma_start(
                out=x_all[:].rearrange("p (a d) -> p a d", d=D),
                in_=x_ext.rearrange("(a p) d -> p a d", p=128))
            xs2 = singles.tile([128, NBT], F32)
            for t in range(NBT):
                xt = x_all[:, t * 128:(t + 1) * 128]
                sq = scr.tile([128, 128], F32, tag="sq")
                nc.vector.scalar_tensor_tensor(
                    out=sq[:], in0=xt, scalar=1.0, in1=xt,
                    op0=ALU.mult, op1=ALU.mult, accum_out=xs2[:, t:t + 1])
            xnrm = singles.tile([128, NBT], F32)
            nc.scalar.activation(xnrm[:], xs2[:], AF.Sqrt)
            xinv = singles.tile([128, NBT], F32)
            nc.vector.reciprocal(xinv[:], xnrm[:])
            xnT = singles.tile([128, B], BF16)
            for t in range(NBT):
                xt = x_all[:, t * 128:(t + 1) * 128]
                xn = scr.tile([128, 128], BF16, tag="wn")
                nc.vector.tensor_scalar(
                    out=xn[:], in0=xt, scalar1=xinv[:, t:t + 1],
                    scalar2=None, op0=ALU.mult)
                tp = tp_pool.tile([128, 128], BF16)
                nc.tensor.transpose(tp[:], xn[:], ident[:])
                nc.scalar.activation(
                    xnT[:, t * 128:(t + 1) * 128], tp[:], AF.Copy)

            # ---------------- load w shard (scoped pool, freed later) ----
            NFULL = NWT - 1  # 97 full 128-row tiles
            wload_ctx = ExitStack()
            wload = wload_ctx.enter_context(
                tc.tile_pool(name="wload", bufs=1))
            w_all = wload.tile([128, NFULL * 128], F32)
            rows_per_dma = 12 * 128
            ndma = (NFULL * 128) // rows_per_dma
            for i in range(ndma):
                r0 = i * rows_per_dma
                nc.sync.dma_start(
                    out=w_all[:, r0:r0 + rows_per_dma].rearrange(
                        "p (a d) -> p a d", d=D),
                    in_=w_ext[r0:r0 + rows_per_dma, :].rearrange(
                        "(a p) d -> p a d", p=128))
            rem0 = ndma * rows_per_dma
            if rem0 < NFULL * 128:
                nc.sync.dma_start(
                    out=w_all[:, rem0:NFULL * 128].rearrange(
                        "p (a d) -> p a d", d=D),
                    in_=w_ext[rem0:NFULL * 128, :].rearrange(
                        "(a p) d -> p a d", p=128))
            w_tail = wload.tile([128, 128], F32)
            nc.sync.dma_start(
                out=w_tail[:WTAIL, :], in_=w_ext[NFULL * 128:CS, :])

            # ---------------- normalize w, build wnT [128, CS] bf16 ------
            def wtile(t):
                if t < NFULL:
                    return w_all[:, t * 128:(t + 1) * 128], 128
                return w_tail[:WTAIL, :], WTAIL

            ns2 = singles.tile([128, NWT], F32)
            wnrm = singles.tile([128, NWT], F32)
            winv = singles.tile([128, NWT], F32)
            wnT = singles.tile([128, CS], BF16)
            for i in range(0, NWT, 12):
                j = min(i + 12, NWT)
                for t in range(i, j):
                    wt, psz = wtile(t)
                    sq = scr.tile([128, 128], F32, tag="sq")
                    nc.vector.scalar_tensor_tensor(
                        out=sq[:psz, :], in0=wt, scalar=1.0, in1=wt,
                        op0=ALU.mult, op1=ALU.mult,
                        accum_out=ns2[:psz, t:t + 1])
                nc.scalar.activation(wnrm[:, i:j], ns2[:, i:j], AF.Sqrt)
                nc.vector.reciprocal(winv[:, i:j], wnrm[:, i:j])
                for t in range(i, j):
                    wt, psz = wtile(t)
                    wn = scr.tile([128, 128], BF16, tag="wn")
                    nc.vector.tensor_scalar(
                        out=wn[:psz, :], in0=wt, scalar1=winv[:psz, t:t + 1],
                        scalar2=None, op0=ALU.mult)
                    tp = tp_pool.tile([128, 128], BF16)
                    nc.tensor.transpose(
                        tp[:, :psz], wn[:psz, :], ident[:psz, :psz])
                    nc.scalar.activation(
                        wnT[:, t * 128:t * 128 + psz], tp[:, :psz], AF.Copy)
            wload_ctx.close()

            # ---------------- main loop ----------------------------------
            # u-pass engine split: DVE for g0, ACT Square for g1 (bigger),
            # sized so both engines carry similar elementwise load.
            GROUPS_STD = [(0, 10, "dve"), (10, 25, "act")]
            GROUPS_LAST = [(0, 7, "dve"), (7, 13, "act"),
                           (13, 19, "dve"), (19, 25, "act")]
            GW = 15 * CHUNK  # 7500 max group width
            cg = ctx.enter_context(tc.tile_pool(name="cg", bufs=2))
            ug = ctx.enter_context(tc.tile_pool(name="ug", bufs=2))
            vg = ctx.enter_context(tc.tile_pool(name="vg", bufs=2))
            ag = ctx.enter_context(tc.tile_pool(name="ag", bufs=2))
            eg = ctx.enter_context(tc.tile_pool(name="eg", bufs=1))
            rs2 = singles.tile([128, 34], F32)
            rs = singles.tile([128, NBT], F32)
            rsum = singles.tile([128, NBT], F32)
            H = NBT // 2
            rs2col = 0
            for bt in range(NBT):
                lhsT = xnT[:, bt * 128:(bt + 1) * 128]
                groups = GROUPS_LAST if bt == NBT - 1 else GROUPS_STD
                cs, us, vs, as_ = [], [], [], []
                for (c0, c1, eng) in groups:
                    W = (c1 - c0) * CHUNK
                    c_all = cg.tile([128, GW], F16, tag="c")
                    for ck in range(c0, c1):
                        pc = mm_pool.tile([128, CHUNK], F32)
                        nc.tensor.matmul(
                            pc[:], lhsT, wnT[:, ck * CHUNK:(ck + 1) * CHUNK],
                            start=True, stop=True)
                        o = (ck - c0) * CHUNK
                        nc.vector.tensor_copy(c_all[:, o:o + CHUNK], pc[:])
                    u_all = ug.tile([128, GW], F16, tag="u")
                    if eng == "dve":
                        nc.vector.scalar_tensor_tensor(
                            out=u_all[:, :W], in0=c_all[:, :W], scalar=1.0,
                            in1=c_all[:, :W], op0=ALU.mult, op1=ALU.mult)
                    else:
                        nc.scalar.activation(
                            u_all[:, :W], c_all[:, :W], AF.Square)
                    cs.append((c_all, W)); us.append(u_all)
                for g in range(len(groups)):
                    (c_all, W), u_all = cs[g], us[g]
                    v_all = vg.tile([128, GW], F16, tag="v")
                    nc.scalar.activation(
                        v_all[:, :W], u_all[:, :W], AF.Sqrt,
                        scale=-K2SQ, bias=k2sqb[:])
                    vs.append(v_all)
                for g in range(len(groups)):
                    (c_all, W), v_all = cs[g], vs[g]
                    a_all = ag.tile([128, GW], F16, tag="a")
                    nc.vector.scalar_tensor_tensor(
                        out=a_all[:, :W], in0=c_all[:, :W], scalar=K1,
                        in1=v_all[:, :W], op0=ALU.mult, op1=ALU.subtract)
                    as_.append(a_all)
                for g in range(len(groups)):
                    (c_all, W), a_all = cs[g], as_[g]
                    e_all = eg.tile([128, GW], BF16, tag="e")
                    nc.scalar.activation(
                        e_all[:, :W], a_all[:, :W], AF.Exp,
                        accum_out=rs2[:, rs2col:rs2col + 1])
                    rs2col += 1

                if bt == H - 1:
                    # first half done: kick AllReduce #1 under the
                    # second half's compute
                    rs2v = rs2[:, :2 * H].rearrange(
                        "p (a two) -> p a two", two=2)
                    nc.vector.tensor_tensor(
                        rs[:, :H], rs2v[:, :, 0], rs2v[:, :, 1], op=ALU.add)
                    rs_in0 = dram.tile([128, H], F32, tag="rsin")
                    rs_out0 = dram.tile([128, H], F32, tag="rsout")
                    nc.sync.dma_start(rs_in0[:], rs[:, :H])
                    nc.gpsimd.collective_compute(
                        "AllReduce", ALU.add,
                        replica_groups=[list(range(NCORES))],
                        ins=[rs_in0.opt()], outs=[rs_out0.opt()])
                    nc.sync.dma_start(rsum[:, :H], rs_out0[:])

            # second-half row sums: bts 8..14 (cols 16..29) + bt15 (30..33)
            rs2v = rs2[:, 16:30].rearrange("p (a two) -> p a two", two=2)
            nc.vector.tensor_tensor(
                rs[:, H:NBT - 1], rs2v[:, :, 0], rs2v[:, :, 1], op=ALU.add)
            nc.vector.tensor_reduce(
                rs[:, NBT - 1:NBT],
                rs2[:, 30:34].rearrange("p (a b) -> p a b", a=1),
                axis=mybir.AxisListType.X, op=ALU.add)
            rs_in1 = dram.tile([128, H], F32, tag="rsin")
            rs_out1 = dram.tile([128, H], F32, tag="rsout")
            nc.sync.dma_start(rs_in1[:], rs[:, H:])
            nc.gpsimd.collective_compute(
                "AllReduce", ALU.add,
                replica_groups=[list(range(NCORES))],
                ins=[rs_in1.opt()], outs=[rs_out1.opt()])
            nc.sync.dma_start(rsum[:, H:], rs_out1[:])

            # ---------------- exact target-term correction (f32) ---------
            # (independent of rsum until s0 — overlaps the main loop)
            wtg_all = singles.tile([128, B], F32)
            nc.sync.dma_start(
                out=wtg_all[:].rearrange("p (a d) -> p a d", d=D),
                in_=wtg_ext.rearrange("(a p) d -> p a d", p=128))
            ws2 = singles.tile([128, NBT], F32)
            dots = singles.tile([128, NBT], F32)
            for t in range(NBT):
                gt = wtg_all[:, t * 128:(t + 1) * 128]
                xt = x_all[:, t * 128:(t + 1) * 128]
                sq = scr.tile([128, 128], F32, tag="sq")
                nc.vector.scalar_tensor_tensor(
                    out=sq[:], in0=gt, scalar=1.0, in1=gt,
                    op0=ALU.mult, op1=ALU.mult, accum_out=ws2[:, t:t + 1])
                dt_ = scr.tile([128, 128], F32, tag="sq")
                nc.vector.tensor_tensor(dt_[:], gt, xt, op=ALU.mult)
                nc.vector.tensor_reduce(
                    dots[:, t:t + 1], dt_[:], axis=mybir.AxisListType.XYZW,
                    op=ALU.add)
            wgn = singles.tile([128, NBT], F32)
            nc.scalar.activation(wgn[:], ws2[:], AF.Sqrt)
            wgi = singles.tile([128, NBT], F32)
            nc.vector.reciprocal(wgi[:], wgn[:])

            ct0 = singles.tile([128, NBT], F32)
            nc.vector.tensor_tensor(ct0[:], dots[:], xinv[:], op=ALU.mult)
            ct = singles.tile([128, NBT], F32)
            nc.vector.tensor_tensor(ct[:], ct0[:], wgi[:], op=ALU.mult)

            u2 = singles.tile([128, NBT], F32)
            nc.vector.tensor_tensor(u2[:], ct[:], ct[:], op=ALU.mult)
            v2 = singles.tile([128, NBT], F32)
            nc.scalar.activation(v2[:], u2[:], AF.Sqrt, scale=-K2SQ,
                                 bias=k2sqb[:])
            a2 = singles.tile([128, NBT], F32)
            nc.vector.scalar_tensor_tensor(
                out=a2[:], in0=ct[:], scalar=K1, in1=v2[:],
                op0=ALU.mult, op1=ALU.subtract)
            t1 = singles.tile([128, NBT], F32)
            nc.scalar.activation(t1[:], a2[:], AF.Exp)
            a3 = singles.tile([128, NBT], F32)
            nc.vector.tensor_scalar(
                out=a3[:], in0=ct[:], scalar1=SCALE, scalar2=-SCALE * MM,
                op0=ALU.mult, op1=ALU.add)
            t2 = singles.tile([128, NBT], F32)
            nc.scalar.activation(t2[:], a3[:], AF.Exp)

            s0 = singles.tile([128, NBT], F32)
            nc.vector.tensor_tensor(s0[:], rsum[:], t1[:], op=ALU.subtract)
            s1 = singles.tile([128, NBT], F32)
            nc.vector.tensor_tensor(s1[:], s0[:], t2[:], op=ALU.add)
            lse = singles.tile([128, NBT], F32)
            nc.scalar.activation(lse[:], s1[:], AF.Ln)
            loss = singles.tile([128, NBT], F32)
            nc.vector.tensor_tensor(loss[:], lse[:], a3[:], op=ALU.subtract)

            lscr = singles.tile([128, NBT], F32)
            lcol = singles.tile([128, 1], F32)
            nc.scalar.activation(
                lscr[:], loss[:], AF.Identity, scale=1.0 / B,
                accum_out=lcol[:])
            fin = fin_pool.tile([1, 1], F32)
            nc.tensor.matmul(fin[:1, :1], ones[:], lcol[:],
                             start=True, stop=True)
            out_sb = singles.tile([1, 1], F32)
            nc.scalar.activation(out_sb[:1, :1], fin[:1, :1], AF.Copy)
            nc.sync.dma_start(out_ext[:, :], out_sb[:1, :1])

    nc.finalize()
    return nc


def _get_nc():
    global _NC
    if _NC is None:
        _NC = _build()
    return _NC


def _in_maps(inputs):
    x = np.ascontiguousarray(np.asarray(inputs["x"], dtype=np.float32))
    target = np.asarray(inputs["target"]).astype(np.int64)
    weight = np.ascontiguousarray(
        np.asarray(inputs["weight"], dtype=np.float32))
    wtg = np.ascontiguousarray(weight[target])
    maps = []
    for c in range(NCORES):
        shard = np.ascontiguousarray(weight[c * CS:(c + 1) * CS])
        maps.append({"x": x, "w": shard, "wtg": wtg})
    return maps


def run(inputs, trace=False, **kw):
    res = run_bass_kernel_spmd(
        _get_nc(), _in_maps(inputs), core_ids=list(range(NCORES)),
        trace=trace, **kw)
    out = np.asarray(res.results[0]["out"], dtype=np.float32).reshape(())
    return out, res


def kernel(**inputs):
    out, _ = run(inputs, trace=False)
    return out


# revision 16
# speedup vs baseline: 1.5233x; 1.0965x over previous
"""ArcFace loss on 8 TRN2 NeuronCores — class-axis (vocab) parallel.

Full inputs in, full scalar loss out. Classes sharded 12500/core; x and the
gathered target weight rows are replicated. Per-core: normalize, bf16 matmul
x_norm @ w_normT, phase-batched exp(s*cos(theta+m)) epilogue with row-sum
accumulation (u-pass split DVE/ACT for engine balance, ACT ops ordered to
minimize act-table reloads), two half AllReduces of the row sums (first one
overlaps the second half of compute), then an exact f32 target-term
correction + log + mean computed redundantly on every core.
"""

import math
from contextlib import ExitStack

import numpy as np

import concourse.bass as bass
import concourse.tile as tile
from concourse import bacc, masks, mybir
from concourse.bass_utils import run_bass_kernel_spmd

B = 2048
D = 128
C = 100000
NCORES = 8
CS = C // NCORES          # 12500 classes per core
NBT = B // 128            # 16 batch tiles
NWT = (CS + 127) // 128   # 98 class tiles (last one is 84 rows)
WTAIL = CS - (NWT - 1) * 128  # 84
CHUNK = 500               # main-loop free-dim chunk (fits one PSUM bank)
NCK = CS // CHUNK         # 25

MARGIN = 0.5
SCALE = 70.0
COS_M = math.cos(MARGIN)
SIN_M = math.sin(MARGIN)
MM = math.sin(math.pi - MARGIN) * MARGIN
K1 = SCALE * COS_M        # 61.43...
K2 = SCALE * SIN_M        # 33.56...
K2SQ = K2 * K2

F32 = mybir.dt.float32
F16 = mybir.dt.float16
BF16 = mybir.dt.bfloat16
AF = mybir.ActivationFunctionType
ALU = mybir.AluOpType

_NC = None


def _build():
    nc = bacc.Bacc(
        "TRN2", target_bir_lowering=False, debug=False, num_devices=NCORES)
    x_ext = nc.declare_dram_parameter("x", [B, D], F32, isOutput=False)
    w_ext = nc.declare_dram_parameter("w", [CS, D], F32, isOutput=False)
    wtg_ext = nc.declare_dram_parameter("wtg", [B, D], F32, isOutput=False)
    out_ext = nc.declare_dram_parameter("out", [1, 1], F32, isOutput=True)

    with tile.TileContext(nc) as tc:
        with ExitStack() as ctx:
            singles = ctx.enter_context(tc.tile_pool(name="singles", bufs=1))
            scr = ctx.enter_context(tc.tile_pool(name="scr", bufs=3))
            mm_pool = ctx.enter_context(
                tc.tile_pool(name="mm", bufs=4, space="PSUM"))
            tp_pool = ctx.enter_context(
                tc.tile_pool(name="tp", bufs=2, space="PSUM"))
            fin_pool = ctx.enter_context(
                tc.tile_pool(name="fin", bufs=1, space="PSUM"))
            dram = ctx.enter_context(
                tc.tile_pool(name="dram", bufs=2, space="DRAM"))

            ident = singles.tile([128, 128], BF16)
            masks.make_identity(nc, ident[:])
            ones = singles.tile([128, 1], F32)
            nc.gpsimd.memset(ones[:], 1.0)
            k2sqb = singles.tile([128, 1], F32)
            nc.gpsimd.memset(k2sqb[:], K2SQ)

            # ---------------- x path first (short pole to first matmul) --
            x_all = singles.tile([128, B], F32)      # col bt*128+d
            nc.sync.dma_start(
                out=x_all[:].rearrange("p (a d) -> p a d", d=D),
                in_=x_ext.rearrange("(a p) d -> p a d", p=128))
            xs2 = singles.tile([128, NBT], F32)
            for t in range(NBT):
                xt = x_all[:, t * 128:(t + 1) * 128]
                sq = scr.tile([128, 128], F32, tag="sq")
                nc.vector.scalar_tensor_tensor(
                    out=sq[:], in0=xt, scalar=1.0, in1=xt,
                    op0=ALU.mult, op1=ALU.mult, accum_out=xs2[:, t:t + 1])
            xnrm = singles.tile([128, NBT], F32)
            nc.scalar.activation(xnrm[:], xs2[:], AF.Sqrt)
            xinv = singles.tile([128, NBT], F32)
            nc.vector.reciprocal(xinv[:], xnrm[:])
            xnT = singles.tile([128, B], BF16)
            for t in range(NBT):
                xt = x_all[:, t * 128:(t + 1) * 128]
                xn = scr.tile([128, 128], BF16, tag="wn")
                nc.vector.tensor_scalar(
                    out=xn[:], in0=xt, scalar1=xinv[:, t:t + 1],
                    scalar2=None, op0=ALU.mult)
                tp = tp_pool.tile([128, 128], BF16)
                nc.tensor.transpose(tp[:], xn[:], ident[:])
                nc.scalar.activation(
                    xnT[:, t * 128:(t + 1) * 128], tp[:], AF.Copy)

            # ---------------- load w shard (scoped pool, freed later) ----
            NFULL = NWT - 1  # 97 full 128-row tiles
            wload_ctx = ExitStack()
            wload = wload_ctx.enter_context(
                tc.tile_pool(name="wload", bufs=1))
            w_all = wload.tile([128, NFULL * 128], F32)
            rows_per_dma = 12 * 128
            ndma = (NFULL * 128) // rows_per_dma
            for i in range(ndma):
                r0 = i * rows_per_dma
                nc.sync.dma_start(
                    out=w_all[:, r0:r0 + rows_per_dma].rearrange(
                        "p (a d) -> p a d", d=D),
                    in_=w_ext[r0:r0 + rows_per_dma, :].rearrange(
                        "(a p) d -> p a d", p=128))
            rem0 = ndma * rows_per_dma
            if rem0 < NFULL * 128:
                nc.sync.dma_start(
                    out=w_all[:, rem0:NFULL * 128].rearrange(
                        "p (a d) -> p a d", d=D),
                    in_=w_ext[rem0:NFULL * 128, :].rearrange(
                        "(a p) d -> p a d", p=128))
            w_tail = wload.tile([128, 128], F32)
            nc.sync.dma_start(
                out=w_tail[:WTAIL, :], in_=w_ext[NFULL * 128:CS, :])

            # ---------------- normalize w, build wnT [128, CS] bf16 ------
            def wtile(t):
                if t < NFULL:
                    return w_all[:, t * 128:(t + 1) * 128], 128
                return w_tail[:WTAIL, :], WTAIL

            ns2 = singles.tile([128, NWT], F32)
            wnrm = singles.tile([128, NWT], F32)
            winv = singles.tile([128, NWT], F32)
            wnT = singles.tile([128, CS], BF16)
            for i in range(0, NWT, 12):
                j = min(i + 12, NWT)
                for t in range(i, j):
                    wt, psz = wtile(t)
                    sq = scr.tile([128, 128], F32, tag="sq")
                    nc.vector.scalar_tensor_tensor(
                        out=sq[:psz, :], in0=wt, scalar=1.0, in1=wt,
                        op0=ALU.mult, op1=ALU.mult,
                        accum_out=ns2[:psz, t:t + 1])
                nc.scalar.activation(wnrm[:, i:j], ns2[:, i:j], AF.Sqrt)
                nc.vector.reciprocal(winv[:, i:j], wnrm[:, i:j])
                for t in range(i, j):
                    wt, psz = wtile(t)
                    wn = scr.tile([128, 128], BF16, tag="wn")
                    nc.vector.tensor_scalar(
                        out=wn[:psz, :], in0=wt, scalar1=winv[:psz, t:t + 1],
                        scalar2=None, op0=ALU.mult)
                    tp = tp_pool.tile([128, 128], BF16)
                    nc.tensor.transpose(
                        tp[:, :psz], wn[:psz, :], ident[:psz, :psz])
                    nc.scalar.activation(
                        wnT[:, t * 128:t * 128 + psz], tp[:, :psz], AF.Copy)
            wload_ctx.close()

            # ---------------- main loop ----------------------------------
            # u-pass engine split: DVE for g0, ACT Square for g1 (bigger),
            # sized so both engines carry similar elementwise load.
            GROUPS_STD = [(0, 10, "dve"), (10, 25, "act")]
            GROUPS_LAST = [(0, 7, "dve"), (7, 13, "act"),
                           (13, 19, "dve"), (19, 25, "act")]
            GW = 15 * CHUNK  # 7500 max group width
            cg = ctx.enter_context(tc.tile_pool(name="cg", bufs=2))
            ug = ctx.enter_context(tc.tile_pool(name="ug", bufs=2))
            vg = ctx.enter_context(tc.tile_pool(name="vg", bufs=2))
            ag = ctx.enter_context(tc.tile_pool(name="ag", bufs=2))
            eg = ctx.enter_context(tc.tile_pool(name="eg", bufs=1))
            rs2 = singles.tile([128, 34], F32)
            rs = singles.tile([128, NBT], F32)
            rsum = singles.tile([128, NBT], F32)
            H = NBT // 2
            rs2col = 0
            for bt in range(NBT):
                lhsT = xnT[:, bt * 128:(bt + 1) * 128]
                groups = GROUPS_LAST if bt == NBT - 1 else GROUPS_STD
                cs, us, vs, as_ = [], [], [], []
                for (c0, c1, eng) in groups:
                    W = (c1 - c0) * CHUNK
                    c_all = cg.tile([128, GW], BF16, tag="c")
                    for ck in range(c0, c1):
                        pc = mm_pool.tile([128, CHUNK], F32)
                        nc.tensor.matmul(
                            pc[:], lhsT, wnT[:, ck * CHUNK:(ck + 1) * CHUNK],
                            start=True, stop=True)
                        o = (ck - c0) * CHUNK
                        nc.vector.tensor_scalar(
                            out=c_all[:, o:o + CHUNK], in0=pc[:],
                            scalar1=K1, scalar2=None, op0=ALU.mult)
                    u_all = ug.tile([128, GW], BF16, tag="u")
                    if eng == "dve":
                        nc.vector.tensor_tensor(
                            u_all[:, :W], c_all[:, :W], c_all[:, :W],
                            op=ALU.mult)
                    else:
                        nc.scalar.activation(
                            u_all[:, :W], c_all[:, :W], AF.Square)
                    cs.append((c_all, W)); us.append(u_all)
                for g in range(len(groups)):
                    (c_all, W), u_all = cs[g], us[g]
                    v_all = vg.tile([128, GW], BF16, tag="v")
                    nc.scalar.activation(
                        v_all[:, :W], u_all[:, :W], AF.Sqrt,
                        scale=-K2SQ / (K1 * K1), bias=k2sqb[:])
                    vs.append(v_all)
                for g in range(len(groups)):
                    (c_all, W), v_all = cs[g], vs[g]
                    a_all = ag.tile([128, GW], BF16, tag="a")
                    nc.vector.tensor_tensor(
                        a_all[:, :W], c_all[:, :W], v_all[:, :W],
                        op=ALU.subtract)
                    as_.append(a_all)
                for g in range(len(groups)):
                    (c_all, W), a_all = cs[g], as_[g]
                    e_all = eg.tile([128, GW], BF16, tag="e")
                    nc.scalar.activation(
                        e_all[:, :W], a_all[:, :W], AF.Exp,
                        accum_out=rs2[:, rs2col:rs2col + 1])
                    rs2col += 1

                if bt == H - 1:
                    # first half done: kick AllReduce #1 under the
                    # second half's compute
                    rs2v = rs2[:, :2 * H].rearrange(
                        "p (a two) -> p a two", two=2)
                    nc.vector.tensor_tensor(
                        rs[:, :H], rs2v[:, :, 0], rs2v[:, :, 1], op=ALU.add)
                    rs_in0 = dram.tile([128, H], F32, tag="rsin")
                    rs_out0 = dram.tile([128, H], F32, tag="rsout")
                    nc.sync.dma_start(rs_in0[:], rs[:, :H])
                    nc.gpsimd.collective_compute(
                        "AllReduce", ALU.add,
                        replica_groups=[list(range(NCORES))],
                        ins=[rs_in0.opt()], outs=[rs_out0.opt()])
                    nc.sync.dma_start(rsum[:, :H], rs_out0[:])

            # second-half row sums: bts 8..14 (cols 16..29) + bt15 (30..33)
            rs2v = rs2[:, 16:30].rearrange("p (a two) -> p a two", two=2)
            nc.vector.tensor_tensor(
                rs[:, H:NBT - 1], rs2v[:, :, 0], rs2v[:, :, 1], op=ALU.add)
            nc.vector.tensor_reduce(
                rs[:, NBT - 1:NBT],
                rs2[:, 30:34].rearrange("p (a b) -> p a b", a=1),
                axis=mybir.AxisListType.X, op=ALU.add)
            rs_in1 = dram.tile([128, H], F32, tag="rsin")
            rs_out1 = dram.tile([128, H], F32, tag="rsout")
            nc.sync.dma_start(rs_in1[:], rs[:, H:])
            nc.gpsimd.collective_compute(
                "AllReduce", ALU.add,
                replica_groups=[list(range(NCORES))],
                ins=[rs_in1.opt()], outs=[rs_out1.opt()])
            nc.sync.dma_start(rsum[:, H:], rs_out1[:])

            # ---------------- exact target-term correction (f32) ---------
            # (independent of rsum until s0 — overlaps the main loop)
            wtg_all = singles.tile([128, B], F32)
            nc.sync.dma_start(
                out=wtg_all[:].rearrange("p (a d) -> p a d", d=D),
                in_=wtg_ext.rearrange("(a p) d -> p a d", p=128))
            ws2 = singles.tile([128, NBT], F32)
            dots = singles.tile([128, NBT], F32)
            for t in range(NBT):
                gt = wtg_all[:, t * 128:(t + 1) * 128]
                xt = x_all[:, t * 128:(t + 1) * 128]
                sq = scr.tile([128, 128], F32, tag="sq")
                nc.vector.scalar_tensor_tensor(
                    out=sq[:], in0=gt, scalar=1.0, in1=gt,
                    op0=ALU.mult, op1=ALU.mult, accum_out=ws2[:, t:t + 1])
                dt_ = scr.tile([128, 128], F32, tag="sq")
                nc.vector.tensor_tensor(dt_[:], gt, xt, op=ALU.mult)
                nc.vector.tensor_reduce(
                    dots[:, t:t + 1], dt_[:], axis=mybir.AxisListType.XYZW,
                    op=ALU.add)
            wgn = singles.tile([128, NBT], F32)
            nc.scalar.activation(wgn[:], ws2[:], AF.Sqrt)
            wgi = singles.tile([128, NBT], F32)
            nc.vector.reciprocal(wgi[:], wgn[:])

            ct0 = singles.tile([128, NBT], F32)
            nc.vector.tensor_tensor(ct0[:], dots[:], xinv[:], op=ALU.mult)
            ct = singles.tile([128, NBT], F32)
            nc.vector.tensor_tensor(ct[:], ct0[:], wgi[:], op=ALU.mult)

            u2 = singles.tile([128, NBT], F32)
            nc.vector.tensor_tensor(u2[:], ct[:], ct[:], op=ALU.mult)
            v2 = singles.tile([128, NBT], F32)
            nc.scalar.activation(v2[:], u2[:], AF.Sqrt, scale=-K2SQ,
                                 bias=k2sqb[:])
            a2 = singles.tile([128, NBT], F32)
            nc.vector.scalar_tensor_tensor(
                out=a2[:], in0=ct[:], scalar=K1, in1=v2[:],
                op0=ALU.mult, op1=ALU.subtract)
            t1 = singles.tile([128, NBT], F32)
            nc.scalar.activation(t1[:], a2[:], AF.Exp)
            a3 = singles.tile([128, NBT], F32)
            nc.vector.tensor_scalar(
                out=a3[:], in0=ct[:], scalar1=SCALE, scalar2=-SCALE * MM,
                op0=ALU.mult, op1=ALU.add)
            t2 = singles.tile([128, NBT], F32)
            nc.scalar.activation(t2[:], a3[:], AF.Exp)

            s0 = singles.tile([128, NBT], F32)
            nc.vector.tensor_tensor(s0[:], rsum[:], t1[:], op=ALU.subtract)
            s1 = singles.tile([128, NBT], F32)
            nc.vector.tensor_tensor(s1[:], s0[:], t2[:], op=ALU.add)
            lse = singles.tile([128, NBT], F32)
            nc.scalar.activation(lse[:], s1[:], AF.Ln)
            loss = singles.tile([128, NBT], F32)
            nc.vector.tensor_tensor(loss[:], lse[:], a3[:], op=ALU.subtract)

            lscr = singles.tile([128, NBT], F32)
            lcol = singles.tile([128, 1], F32)
            nc.scalar.activation(
                lscr[:], loss[:], AF.Identity, scale=1.0 / B,
                accum_out=lcol[:])
            fin = fin_pool.tile([1, 1], F32)
            nc.tensor.matmul(fin[:1, :1], ones[:], lcol[:],
                             start=True, stop=True)
            out_sb = singles.tile([1, 1], F32)
            nc.scalar.activation(out_sb[:1, :1], fin[:1, :1], AF.Copy)
            nc.sync.dma_start(out_ext[:, :], out_sb[:1, :1])

    nc.finalize()
    return nc


def _get_nc():
    global _NC
    if _NC is None:
        _NC = _build()
    return _NC


def _in_maps(inputs):
    x = np.ascontiguousarray(np.asarray(inputs["x"], dtype=np.float32))
    target = np.asarray(inputs["target"]).astype(np.int64)
    weight = np.ascontiguousarray(
        np.asarray(inputs["weight"], dtype=np.float32))
    wtg = np.ascontiguousarray(weight[target])
    maps = []
    for c in range(NCORES):
        shard = np.ascontiguousarray(weight[c * CS:(c + 1) * CS])
        maps.append({"x": x, "w": shard, "wtg": wtg})
    return maps


def run(inputs, trace=False, **kw):
    res = run_bass_kernel_spmd(
        _get_nc(), _in_maps(inputs), core_ids=list(range(NCORES)),
        trace=trace, **kw)
    out = np.asarray(res.results[0]["out"], dtype=np.float32).reshape(())
    return out, res


def kernel(**inputs):
    out, _ = run(inputs, trace=False)
    return out


# revision 17
# speedup vs baseline: 1.6359x; 1.0739x over previous
"""ArcFace loss on 8 TRN2 NeuronCores — class-axis (vocab) parallel.

Full inputs in, full scalar loss out. Classes sharded 12500/core; x and the
gathered target weight rows are replicated. Per-core: normalize, bf16 matmul
x_norm @ w_normT, phase-batched exp(s*cos(theta+m)) epilogue with row-sum
accumulation (u-pass split DVE/ACT for engine balance, ACT ops ordered to
minimize act-table reloads), two half AllReduces of the row sums (first one
overlaps the second half of compute), then an exact f32 target-term
correction + log + mean computed redundantly on every core.
"""

import math
from contextlib import ExitStack

import numpy as np

import concourse.bass as bass
import concourse.tile as tile
from concourse import bacc, masks, mybir
from concourse.bass_utils import run_bass_kernel_spmd

B = 2048
D = 128
C = 100000
NCORES = 8
CS = C // NCORES          # 12500 classes per core
NBT = B // 128            # 16 batch tiles
NWT = (CS + 127) // 128   # 98 class tiles (last one is 84 rows)
WTAIL = CS - (NWT - 1) * 128  # 84
CHUNK = 500               # main-loop free-dim chunk (fits one PSUM bank)
NCK = CS // CHUNK         # 25

MARGIN = 0.5
SCALE = 70.0
COS_M = math.cos(MARGIN)
SIN_M = math.sin(MARGIN)
MM = math.sin(math.pi - MARGIN) * MARGIN
K1 = SCALE * COS_M        # 61.43...
K2 = SCALE * SIN_M        # 33.56...
K2SQ = K2 * K2

F32 = mybir.dt.float32
F16 = mybir.dt.float16
BF16 = mybir.dt.bfloat16
AF = mybir.ActivationFunctionType
ALU = mybir.AluOpType

_NC = None


def _build():
    nc = bacc.Bacc(
        "TRN2", target_bir_lowering=False, debug=False, num_devices=NCORES)
    x_ext = nc.declare_dram_parameter("x", [B, D], F32, isOutput=False)
    w_ext = nc.declare_dram_parameter("w", [CS, D], F32, isOutput=False)
    wtg_ext = nc.declare_dram_parameter("wtg", [B, D], F32, isOutput=False)
    out_ext = nc.declare_dram_parameter("out", [1, 1], F32, isOutput=True)

    with tile.TileContext(nc) as tc:
        with ExitStack() as ctx:
            singles = ctx.enter_context(tc.tile_pool(name="singles", bufs=1))
            scr = ctx.enter_context(tc.tile_pool(name="scr", bufs=3))
            mm_pool = ctx.enter_context(
                tc.tile_pool(name="mm", bufs=4, space="PSUM"))
            tp_pool = ctx.enter_context(
                tc.tile_pool(name="tp", bufs=2, space="PSUM"))
            fin_pool = ctx.enter_context(
                tc.tile_pool(name="fin", bufs=1, space="PSUM"))
            dram = ctx.enter_context(
                tc.tile_pool(name="dram", bufs=2, space="DRAM"))

            ident = singles.tile([128, 128], BF16)
            masks.make_identity(nc, ident[:])
            ones = singles.tile([128, 1], F32)
            nc.gpsimd.memset(ones[:], 1.0)
            k2sqb = singles.tile([128, 1], F32)
            nc.gpsimd.memset(k2sqb[:], K2SQ)

            # ---------------- x path first (short pole to first matmul) --
            x_all = singles.tile([128, B], F32)      # col bt*128+d
            nc.sync.dma_start(
                out=x_all[:].rearrange("p (a d) -> p a d", d=D),
                in_=x_ext.rearrange("(a p) d -> p a d", p=128))
            xs2 = singles.tile([128, NBT], F32)
            for t in range(NBT):
                xt = x_all[:, t * 128:(t + 1) * 128]
                sq = scr.tile([128, 128], F32, tag="sq")
                nc.vector.scalar_tensor_tensor(
                    out=sq[:], in0=xt, scalar=1.0, in1=xt,
                    op0=ALU.mult, op1=ALU.mult, accum_out=xs2[:, t:t + 1])
            xnrm = singles.tile([128, NBT], F32)
            nc.scalar.activation(xnrm[:], xs2[:], AF.Sqrt)
            xinv = singles.tile([128, NBT], F32)
            nc.vector.reciprocal(xinv[:], xnrm[:])
            xnT = singles.tile([128, B], BF16)
            for t in range(NBT):
                xt = x_all[:, t * 128:(t + 1) * 128]
                xn = scr.tile([128, 128], BF16, tag="wn")
                nc.vector.tensor_scalar(
                    out=xn[:], in0=xt, scalar1=xinv[:, t:t + 1],
                    scalar2=None, op0=ALU.mult)
                tp = tp_pool.tile([128, 128], BF16)
                nc.tensor.transpose(tp[:], xn[:], ident[:])
                nc.vector.tensor_copy(
                    xnT[:, t * 128:(t + 1) * 128], tp[:])

            # ---------------- load w shard (scoped pool, freed later) ----
            NFULL = NWT - 1  # 97 full 128-row tiles
            wload_ctx = ExitStack()
            wload = wload_ctx.enter_context(
                tc.tile_pool(name="wload", bufs=1))
            w_all = wload.tile([128, NFULL * 128], F32)
            rows_per_dma = 12 * 128
            ndma = (NFULL * 128) // rows_per_dma
            for i in range(ndma):
                r0 = i * rows_per_dma
                nc.sync.dma_start(
                    out=w_all[:, r0:r0 + rows_per_dma].rearrange(
                        "p (a d) -> p a d", d=D),
                    in_=w_ext[r0:r0 + rows_per_dma, :].rearrange(
                        "(a p) d -> p a d", p=128))
            rem0 = ndma * rows_per_dma
            if rem0 < NFULL * 128:
                nc.sync.dma_start(
                    out=w_all[:, rem0:NFULL * 128].rearrange(
                        "p (a d) -> p a d", d=D),
                    in_=w_ext[rem0:NFULL * 128, :].rearrange(
                        "(a p) d -> p a d", p=128))
            w_tail = wload.tile([128, 128], F32)
            nc.sync.dma_start(
                out=w_tail[:WTAIL, :], in_=w_ext[NFULL * 128:CS, :])

            # ---------------- normalize w, build wnT [128, CS] bf16 ------
            def wtile(t):
                if t < NFULL:
                    return w_all[:, t * 128:(t + 1) * 128], 128
                return w_tail[:WTAIL, :], WTAIL

            ns2 = singles.tile([128, NWT], F32)
            wnrm = singles.tile([128, NWT], F32)
            winv = singles.tile([128, NWT], F32)
            wnT = singles.tile([128, CS], BF16)
            for i in range(0, NWT, 12):
                j = min(i + 12, NWT)
                for t in range(i, j):
                    wt, psz = wtile(t)
                    sq = scr.tile([128, 128], F32, tag="sq")
                    nc.vector.scalar_tensor_tensor(
                        out=sq[:psz, :], in0=wt, scalar=1.0, in1=wt,
                        op0=ALU.mult, op1=ALU.mult,
                        accum_out=ns2[:psz, t:t + 1])
                nc.scalar.activation(wnrm[:, i:j], ns2[:, i:j], AF.Sqrt)
                nc.vector.reciprocal(winv[:, i:j], wnrm[:, i:j])
                for t in range(i, j):
                    wt, psz = wtile(t)
                    wn = scr.tile([128, 128], BF16, tag="wn")
                    nc.vector.tensor_scalar(
                        out=wn[:psz, :], in0=wt, scalar1=winv[:psz, t:t + 1],
                        scalar2=K1, op0=ALU.mult, op1=ALU.mult)
                    tp = tp_pool.tile([128, 128], BF16)
                    nc.tensor.transpose(
                        tp[:, :psz], wn[:psz, :], ident[:psz, :psz])
                    nc.vector.tensor_copy(
                        wnT[:, t * 128:t * 128 + psz], tp[:, :psz])
            wload_ctx.close()

            # ---------------- main loop ----------------------------------
            # u-pass engine split: DVE for g0, ACT Square for g1 (bigger),
            # sized so both engines carry similar elementwise load.
            GROUPS_STD = [(0, 10, "dve"), (10, 25, "act")]
            GROUPS_LAST = [(0, 7, "dve"), (7, 13, "act"),
                           (13, 19, "dve"), (19, 25, "act")]
            GW = 15 * CHUNK  # 7500 max group width
            cg = ctx.enter_context(tc.tile_pool(name="cg", bufs=2))
            ug = ctx.enter_context(tc.tile_pool(name="ug", bufs=2))
            vg = ctx.enter_context(tc.tile_pool(name="vg", bufs=2))
            ag = ctx.enter_context(tc.tile_pool(name="ag", bufs=2))
            eg = ctx.enter_context(tc.tile_pool(name="eg", bufs=1))
            rs2 = singles.tile([128, 34], F32)
            rs = singles.tile([128, NBT], F32)
            rsum = singles.tile([128, NBT], F32)
            H = NBT // 2
            rs2col = 0
            for bt in range(NBT):
                lhsT = xnT[:, bt * 128:(bt + 1) * 128]
                groups = GROUPS_LAST if bt == NBT - 1 else GROUPS_STD
                cs, us, vs, as_ = [], [], [], []
                for (c0, c1, eng) in groups:
                    W = (c1 - c0) * CHUNK
                    c_all = cg.tile([128, GW], BF16, tag="c")
                    for ck in range(c0, c1):
                        pc = mm_pool.tile([128, CHUNK], F32)
                        nc.tensor.matmul(
                            pc[:], lhsT, wnT[:, ck * CHUNK:(ck + 1) * CHUNK],
                            start=True, stop=True)
                        o = (ck - c0) * CHUNK
                        nc.vector.tensor_copy(c_all[:, o:o + CHUNK], pc[:])
                    u_all = ug.tile([128, GW], BF16, tag="u")
                    nc.vector.tensor_tensor(
                        u_all[:, :W], c_all[:, :W], c_all[:, :W],
                        op=ALU.mult)
                    cs.append((c_all, W)); us.append(u_all)
                for g in range(len(groups)):
                    (c_all, W), u_all = cs[g], us[g]
                    v_all = vg.tile([128, GW], BF16, tag="v")
                    nc.scalar.activation(
                        v_all[:, :W], u_all[:, :W], AF.Sqrt,
                        scale=-K2SQ / (K1 * K1), bias=k2sqb[:])
                    vs.append(v_all)
                for g in range(len(groups)):
                    (c_all, W), v_all = cs[g], vs[g]
                    a_all = ag.tile([128, GW], BF16, tag="a")
                    nc.vector.tensor_tensor(
                        a_all[:, :W], c_all[:, :W], v_all[:, :W],
                        op=ALU.subtract)
                    as_.append(a_all)
                for g in range(len(groups)):
                    (c_all, W), a_all = cs[g], as_[g]
                    e_all = eg.tile([128, GW], BF16, tag="e")
                    nc.scalar.activation(
                        e_all[:, :W], a_all[:, :W], AF.Exp,
                        accum_out=rs2[:, rs2col:rs2col + 1])
                    rs2col += 1

                if bt == H - 1:
                    # first half done: kick AllReduce #1 under the
                    # second half's compute
                    rs2v = rs2[:, :2 * H].rearrange(
                        "p (a two) -> p a two", two=2)
                    nc.vector.tensor_tensor(
                        rs[:, :H], rs2v[:, :, 0], rs2v[:, :, 1], op=ALU.add)
                    rs_in0 = dram.tile([128, H], F32, tag="rsin")
                    rs_out0 = dram.tile([128, H], F32, tag="rsout")
                    nc.sync.dma_start(rs_in0[:], rs[:, :H])
                    nc.gpsimd.collective_compute(
                        "AllReduce", ALU.add,
                        replica_groups=[list(range(NCORES))],
                        ins=[rs_in0.opt()], outs=[rs_out0.opt()])
                    nc.sync.dma_start(rsum[:, :H], rs_out0[:])

            # second-half row sums: bts 8..14 (cols 16..29) + bt15 (30..33)
            rs2v = rs2[:, 16:30].rearrange("p (a two) -> p a two", two=2)
            nc.vector.tensor_tensor(
                rs[:, H:NBT - 1], rs2v[:, :, 0], rs2v[:, :, 1], op=ALU.add)
            nc.vector.tensor_reduce(
                rs[:, NBT - 1:NBT],
                rs2[:, 30:34].rearrange("p (a b) -> p a b", a=1),
                axis=mybir.AxisListType.X, op=ALU.add)
            rs_in1 = dram.tile([128, H], F32, tag="rsin")
            rs_out1 = dram.tile([128, H], F32, tag="rsout")
            nc.sync.dma_start(rs_in1[:], rs[:, H:])
            nc.gpsimd.collective_compute(
                "AllReduce", ALU.add,
                replica_groups=[list(range(NCORES))],
                ins=[rs_in1.opt()], outs=[rs_out1.opt()])
            nc.sync.dma_start(rsum[:, H:], rs_out1[:])

            # ---------------- exact target-term correction (f32) ---------
            # (independent of rsum until s0 — overlaps the main loop)
            wtg_all = singles.tile([128, B], F32)
            nc.sync.dma_start(
                out=wtg_all[:].rearrange("p (a d) -> p a d", d=D),
                in_=wtg_ext.rearrange("(a p) d -> p a d", p=128))
            ws2 = singles.tile([128, NBT], F32)
            dots = singles.tile([128, NBT], F32)
            for t in range(NBT):
                gt = wtg_all[:, t * 128:(t + 1) * 128]
                xt = x_all[:, t * 128:(t + 1) * 128]
                sq = scr.tile([128, 128], F32, tag="sq")
                nc.vector.scalar_tensor_tensor(
                    out=sq[:], in0=gt, scalar=1.0, in1=gt,
                    op0=ALU.mult, op1=ALU.mult, accum_out=ws2[:, t:t + 1])
                dt_ = scr.tile([128, 128], F32, tag="sq")
                nc.vector.tensor_tensor(dt_[:], gt, xt, op=ALU.mult)
                nc.vector.tensor_reduce(
                    dots[:, t:t + 1], dt_[:], axis=mybir.AxisListType.XYZW,
                    op=ALU.add)
            wgn = singles.tile([128, NBT], F32)
            nc.scalar.activation(wgn[:], ws2[:], AF.Sqrt)
            wgi = singles.tile([128, NBT], F32)
            nc.vector.reciprocal(wgi[:], wgn[:])

            ct0 = singles.tile([128, NBT], F32)
            nc.vector.tensor_tensor(ct0[:], dots[:], xinv[:], op=ALU.mult)
            ct = singles.tile([128, NBT], F32)
            nc.vector.tensor_tensor(ct[:], ct0[:], wgi[:], op=ALU.mult)

            u2 = singles.tile([128, NBT], F32)
            nc.vector.tensor_tensor(u2[:], ct[:], ct[:], op=ALU.mult)
            v2 = singles.tile([128, NBT], F32)
            nc.scalar.activation(v2[:], u2[:], AF.Sqrt, scale=-K2SQ,
                                 bias=k2sqb[:])
            a2 = singles.tile([128, NBT], F32)
            nc.vector.scalar_tensor_tensor(
                out=a2[:], in0=ct[:], scalar=K1, in1=v2[:],
                op0=ALU.mult, op1=ALU.subtract)
            t1 = singles.tile([128, NBT], F32)
            nc.scalar.activation(t1[:], a2[:], AF.Exp)
            a3 = singles.tile([128, NBT], F32)
            nc.vector.tensor_scalar(
                out=a3[:], in0=ct[:], scalar1=SCALE, scalar2=-SCALE * MM,
                op0=ALU.mult, op1=ALU.add)
            t2 = singles.tile([128, NBT], F32)
            nc.scalar.activation(t2[:], a3[:], AF.Exp)

            s0 = singles.tile([128, NBT], F32)
            nc.vector.tensor_tensor(s0[:], rsum[:], t1[:], op=ALU.subtract)
            s1 = singles.tile([128, NBT], F32)
            nc.vector.tensor_tensor(s1[:], s0[:], t2[:], op=ALU.add)
            lse = singles.tile([128, NBT], F32)
            nc.scalar.activation(lse[:], s1[:], AF.Ln)
            loss = singles.tile([128, NBT], F32)
            nc.vector.tensor_tensor(loss[:], lse[:], a3[:], op=ALU.subtract)

            lscr = singles.tile([128, NBT], F32)
            lcol = singles.tile([128, 1], F32)
            nc.scalar.activation(
                lscr[:], loss[:], AF.Identity, scale=1.0 / B,
                accum_out=lcol[:])
            fin = fin_pool.tile([1, 1], F32)
            nc.tensor.matmul(fin[:1, :1], ones[:], lcol[:],
                             start=True, stop=True)
            out_sb = singles.tile([1, 1], F32)
            nc.scalar.activation(out_sb[:1, :1], fin[:1, :1], AF.Copy)
            nc.sync.dma_start(out_ext[:, :], out_sb[:1, :1])

    nc.finalize()
    return nc


def _get_nc():
    global _NC
    if _NC is None:
        _NC = _build()
    return _NC


def _in_maps(inputs):
    x = np.ascontiguousarray(np.asarray(inputs["x"], dtype=np.float32))
    target = np.asarray(inputs["target"]).astype(np.int64)
    weight = np.ascontiguousarray(
        np.asarray(inputs["weight"], dtype=np.float32))
    wtg = np.ascontiguousarray(weight[target])
    maps = []
    for c in range(NCORES):
        shard = np.ascontiguousarray(weight[c * CS:(c + 1) * CS])
        maps.append({"x": x, "w": shard, "wtg": wtg})
    return maps


def run(inputs, trace=False, **kw):
    res = run_bass_kernel_spmd(
        _get_nc(), _in_maps(inputs), core_ids=list(range(NCORES)),
        trace=trace, **kw)
    out = np.asarray(res.results[0]["out"], dtype=np.float32).reshape(())
    return out, res


def kernel(**inputs):
    out, _ = run(inputs, trace=False)
    return out
